# revision 14
# baseline (speedup 1.0000x reference)
"""Trainium2 Bass kernel v3 for LowRankOrthogonalProjection.

    out = target + (source - target) @ W @ W.T        (W: [D, R], R=16)

All three big streams travel at 1 byte/element (25.2 MB/core HBM vs 42 MB
in v2):

  * srcp: fp8e4(src/STEP), DoubleRow-packed.  Feeds the PE directly.
  * tgtp: int8 round(tgt/STEP).  SWDGE cast-DMA upcasts to bf16 in SBUF
    (exact small integers), so the same tile serves both the matmul path
    and the final add -- no elementwise sub anywhere.
  * outp: int8.  One DVE op per chunk: out_i8 = rne_sat(corr_psum + tgt_bf16)
    (DVE float->int8 output is round-nearest-even + saturating, HW-verified).

diff = src - tgt is formed *in PSUM* by a two-pass accumulation:
  pass 1: fp8 DoubleRow matmuls   src_u @ (+32 W)
  pass 2: bf16 matmuls            tgt_u @ (-32 W)
tT = 32*(diff_u @ W) -> ACT scales 1/32 -> bf16 -> m2 (wt stationary) ->
corr_u in PSUM -> DVE add+quantize -> int8 out.

The rank-16 row coefficients tT are also written out (tiny) so the host can
exactly patch the ~0.03% of rows where |tgt| > TPATCH would saturate int8.

Layout (per core, rpc=2048 rows in NQ slabs of QR=2048/NQ rows):
  srcp[q*128+p, c2*2*QR + b*QR + r] = fp8(src[q*QR+r, c2*256+2p+b]/STEP)
  tgtp[q*128+p, c*QR+r]             = int8(round(tgt[q*QR+r, c*128+p]/STEP))
  outp: same indexing as tgtp, int8.
  wc8[p, c2*32 + b*16 + j] = fp8(32*W[c2*256+2p+b, j])
  wcn[p, c*16+j]           = bf16(-32*W[c*128+p, j])
  wt[j, d]                 = bf16(W[d, j])
  outt[j, q*QR+r]          = bf16(tT_u[q*QR+r, j])
"""

import numpy as np
import ml_dtypes

B, S, D, R = 4, 4096, 4096, 16
N_CORES = 8
ROWS = B * S                 # 16384
RPC = ROWS // N_CORES        # 2048 rows per core
P = 128
DCH = D // P                 # 32 D-chunks
WSCALE = 32.0                # fp8 weight scale
STEP = 0.0325                # int8 quantization step for tgt/out
TPATCH = 3.6                 # host patches rows with |tgt| > TPATCH
_NC_CACHE = {}

# default build config (kernel() uses this; bench can override)
CFG = dict(nq=4, pb_group=2, tgt_split=2, src_split=2, out_split=2,
           tgt_bufs=2, src_bufs=4, out_bufs=2, out_dma="sync",
           tgt_upcast="act", upcast_split=8, upcast_pool=0, act_groups=0)


def build_nc(reps=1, nq=4, pb_group=2, tgt_split=2, src_split=2, out_split=2,
             tgt_bufs=2, src_bufs=4, out_bufs=2, out_dma="sync",
             tgt_upcast="dma", upcast_split=4, upcast_pool=0, act_groups=0):
    import concourse.bass as bass
    import concourse.mybir as mybir
    import concourse.tile as tile

    bf16 = mybir.dt.bfloat16
    f32 = mybir.dt.float32
    f8 = mybir.dt.float8e4
    i8 = mybir.dt.int8

    QR = RPC // nq           # rows per slab
    QW = DCH * QR            # packed free width
    HW = QW // 2             # half-slab free width
    DC2 = DCH // 2

    nc = bass.Bass("TRN2", target_bir_lowering=False)

    srcp = nc.dram_tensor("srcp", [nq * P, QW], f8, kind="ExternalInput")
    tgtp = nc.dram_tensor("tgtp", [nq * P, QW], i8, kind="ExternalInput")
    wc8 = nc.dram_tensor("wc8", [P, DCH * R], f8, kind="ExternalInput")
    wcn = nc.dram_tensor("wcn", [P, DCH * R], bf16, kind="ExternalInput")
    wt = nc.dram_tensor("wt", [R, D], bf16, kind="ExternalInput")
    outp = nc.dram_tensor("outp", [nq * P, QW], i8, kind="ExternalOutput")
    outt = nc.dram_tensor("outt", [R, RPC], bf16, kind="ExternalOutput")
    ident = nc.dram_tensor("ident", [P, P], bf16, kind="ExternalInput")

    out_eng = dict(sync="sync", scalar="scalar")[out_dma]

    with tile.TileContext(nc) as tc:
        with (
            tc.tile_pool(name="const", bufs=1) as cpool,
            tc.tile_pool(name="tgtp_", bufs=tgt_bufs) as tgt_pool,
            tc.tile_pool(name="tgt8p", bufs=2) as tgt8_pool,
            tc.tile_pool(name="srcp_", bufs=src_bufs) as src_pool,
            tc.tile_pool(name="tmtp", bufs=2) as tmt_pool,
            tc.tile_pool(name="outp_", bufs=out_bufs) as out_pool,
            tc.tile_pool(name="ps_t", bufs=2, space="PSUM") as ps_t,
            tc.tile_pool(name="ps_o", bufs=3, space="PSUM") as ps_o,
        ):
            wc8_sb = cpool.tile([P, DCH * R], f8)
            nc.sync.dma_start(wc8_sb, wc8[:, :])
            wcn_sb = cpool.tile([P, DCH * R], bf16)
            nc.sync.dma_start(wcn_sb, wcn[:, :])
            wt_sb = cpool.tile([R, D], bf16)
            nc.sync.dma_start(wt_sb, wt[:, :])
            if act_groups:
                id_sb = cpool.tile([P, P], bf16)
                nc.sync.dma_start(id_sb, ident[:, :])

            NG = DCH // 2 // pb_group * 2          # psum groups per slab
            act_sel = set(
                round(i * NG / act_groups) % NG for i in range(act_groups)
            ) if act_groups else set()

            def ap3(t2d, ko, dim):
                a = [list(x) for x in t2d.ap]
                return bass.AP(
                    t2d.tensor, t2d.offset,
                    [a[0], [dim, ko], [1, dim]],
                )

            for qi in range(nq * reps):
                q = qi % nq
                qs = q * P

                # tgt: int8 on the wire, bf16 in SBUF
                tgt_sb = tgt_pool.tile([P, QW], bf16, tag="tgt")
                tw = QW // tgt_split
                if tgt_upcast == "dma":
                    # SWDGE cast-DMA (costs 2x bytes on the SBUF fabric side)
                    for ts in range(tgt_split):
                        nc.gpsimd.dma_start(
                            tgt_sb[:, ts * tw : (ts + 1) * tw],
                            tgtp[qs : qs + P, ts * tw : (ts + 1) * tw],
                        )
                else:
                    # plain int8 load + engine upcast (pool: otherwise idle)
                    tgt8_sb = tgt8_pool.tile([P, QW], i8, tag="tgt8")
                    for ts in range(tgt_split):
                        nc.sync.dma_start(
                            tgt8_sb[:, ts * tw : (ts + 1) * tw],
                            tgtp[qs : qs + P, ts * tw : (ts + 1) * tw],
                        )
                    uw = QW // upcast_split
                    for us in range(upcast_split):
                        dst = tgt_sb[:, us * uw : (us + 1) * uw]
                        src_ = tgt8_sb[:, us * uw : (us + 1) * uw]
                        if tgt_upcast == "act":
                            if us < upcast_pool:
                                nc.gpsimd.tensor_copy(dst, src_)
                            else:
                                nc.scalar.copy(dst, src_)
                        elif tgt_upcast == "pool":
                            nc.gpsimd.tensor_copy(dst, src_)
                        else:
                            nc.vector.tensor_copy(dst, src_)

                # src: fp8 straight into SBUF (plain HWDGE)
                if src_split == 1:
                    s = src_pool.tile([P, QW], f8, tag="src")
                    nc.sync.dma_start(s, srcp[qs : qs + P, :])
                    src_sb = [s[:, 0:HW], s[:, HW:QW]]
                else:
                    src_sb = []
                    for h in range(2):
                        s = src_pool.tile([P, HW], f8, tag="src")
                        nc.sync.dma_start(
                            s, srcp[qs : qs + P, h * HW : (h + 1) * HW]
                        )
                        src_sb.append(s)

                # m1: tT_ps = 32*(src_u - tgt_u) @ W, fp8-DR + bf16 two-pass
                tT_ps = ps_t.tile([R, QR], f32, tag="tT")
                first = True
                for h in range(2):
                    for j in range(DC2 // 2):
                        c2 = h * (DC2 // 2) + j
                        nc.tensor.matmul(
                            tT_ps,
                            ap3(wc8_sb[:, c2 * 2 * R : (c2 + 1) * 2 * R], 2, R),
                            ap3(src_sb[h][:, j * 2 * QR : (j + 1) * 2 * QR], 2, QR),
                            start=first,
                            stop=False,
                            perf_mode=mybir.MatmulPerfMode.DoubleRow,
                        )
                        first = False
                    for jc in range(DCH // 2):
                        c = h * (DCH // 2) + jc
                        nc.tensor.matmul(
                            tT_ps,
                            wcn_sb[:, c * R : (c + 1) * R],
                            tgt_sb[:, c * QR : (c + 1) * QR],
                            start=False,
                            stop=(c == DCH - 1),
                        )

                tT_sb = tmt_pool.tile([R, QR], bf16, tag="tTs")
                nc.scalar.mul(tT_sb, tT_ps, 1.0 / WSCALE)
                nc.sync.dma_start(outt[:, q * QR : (q + 1) * QR], tT_sb)

                # m2 + fused add/quantize
                PB = pb_group
                OW = QW // out_split
                out_sb = None
                ob = 0
                gidx = 0
                for h in range(2):
                    if out_sb is None:
                        out_sb = out_pool.tile([P, OW], i8, tag="out")
                        ob = h * (DCH // 2)
                    for j in range(DCH // 2 // PB):
                        use_act = gidx in act_sel
                        gidx += 1
                        o_ps = ps_o.tile([P, PB * QR], f32, tag="ops")
                        for b in range(PB):
                            c = h * (DCH // 2) + j * PB + b
                            nc.tensor.matmul(
                                o_ps[:, b * QR : (b + 1) * QR],
                                wt_sb[:, c * P : (c + 1) * P],
                                tT_sb,
                                start=True,
                                stop=not use_act,
                                skip_group_check=use_act,
                            )
                        c0 = h * (DCH // 2) + j * PB
                        if use_act:
                            # fold tgt into PSUM on the PE, evacuate on ACT
                            for b in range(PB):
                                nc.tensor.matmul(
                                    o_ps[:, b * QR : (b + 1) * QR],
                                    id_sb,
                                    tgt_sb[:, (c0 + b) * QR : (c0 + b + 1) * QR],
                                    start=False,
                                    stop=(b == PB - 1),
                                    skip_group_check=True,
                                )
                            nc.scalar.copy(
                                out_sb[:, (c0 - ob) * QR : (c0 - ob + PB) * QR],
                                o_ps,
                            )
                        else:
                            nc.vector.tensor_add(
                                out_sb[:, (c0 - ob) * QR : (c0 - ob + PB) * QR],
                                o_ps,
                                tgt_sb[:, c0 * QR : (c0 + PB) * QR],
                            )
                    if (h + 1) % (2 // out_split) == 0:
                        getattr(nc, out_eng).dma_start(
                            outp[qs : qs + P, ob * QR : ob * QR + OW], out_sb
                        )
                        out_sb = None

    return nc


def split_waits(nc, limit=1):
    """Walrus encodes at most one semaphore wait per instruction; hoist
    extras onto standalone EventSemaphore instructions."""
    import concourse.mybir as mybir

    nsplit = 0
    for fn in nc.m.functions:
        for blk in fn.blocks:
            new = []
            for ins in blk.instructions:
                si = ins.sync_info
                waits = list(si.on_wait) if si is not None and si.on_wait else []
                if len(waits) > limit:
                    for k, w in enumerate(waits[:-limit]):
                        es = mybir.InstEventSemaphore(
                            name=f"{ins.name}-hw{k}",
                            engine=ins.engine,
                            sync_info=mybir.SyncInfo(on_wait=[w], on_update=[]),
                        )
                        new.append(es)
                        nsplit += 1
                    ins.sync_info = mybir.SyncInfo(
                        on_wait=waits[-limit:],
                        on_update=list(si.on_update or []),
                    )
                new.append(ins)
            blk.instructions[:] = new
    return nsplit


def _get_nc(reps=1, raw=False, **kw):
    cfg = dict(CFG)
    cfg.update(kw)
    key = (reps, raw, tuple(sorted(cfg.items())))
    if key not in _NC_CACHE:
        nc = build_nc(reps, **cfg)
        nc.finalize()
        if not raw:
            split_waits(nc)
        _NC_CACHE[key] = nc
    return _NC_CACHE[key]


def _pack(x2, dtype, nq):
    """[16384, 4096] row-major -> per-core [nq*128, 32*QR] slab-packed."""
    QR = RPC // nq
    xq = np.asarray(x2).astype(dtype)
    xp = (
        xq.reshape(N_CORES, nq, QR, DCH, P)
        .transpose(0, 1, 4, 3, 2)
        .reshape(N_CORES, nq * P, DCH * QR)
    )
    return np.ascontiguousarray(xp)


def _pack_dr(x2, dtype, nq):
    """DoubleRow pack: [q*128+p, c2*2*QR + b*QR + r] = x[q*QR+r, c2*256+2p+b]."""
    QR = RPC // nq
    DC2 = DCH // 2
    xq = np.asarray(x2).astype(dtype)
    xp = (
        xq.reshape(N_CORES, nq, QR, DC2, P, 2)     # (core, q, r, c2, p, b)
        .transpose(0, 1, 4, 3, 5, 2)               # (core, q, p, c2, b, r)
        .reshape(N_CORES, nq * P, DCH * QR)
    )
    return np.ascontiguousarray(xp)


def make_host_inputs(source, target, weight, nq=None):
    import concourse.mybir as mybir

    nq = nq or CFG["nq"]
    f8dt = mybir.dt.np(mybir.dt.float8e4)
    bf = ml_dtypes.bfloat16

    src2 = np.asarray(source, dtype=np.float32).reshape(ROWS, D)
    tgt2 = np.asarray(target, dtype=np.float32).reshape(ROWS, D)
    w = np.asarray(weight, dtype=np.float32)

    srcp = _pack_dr(src2 * np.float32(1.0 / STEP), f8dt, nq)
    tgtq = np.clip(np.rint(tgt2 * np.float32(1.0 / STEP)), -127, 127).astype(
        np.int8
    )
    tgtp = _pack(tgtq, np.int8, nq)
    # wc8[p, c2*32 + b*16 + j] = 32*W[c2*256 + 2p + b, j]
    wc8 = np.ascontiguousarray(
        np.clip(WSCALE * w, -240, 240)
        .reshape(DCH // 2, P, 2, R)
        .transpose(1, 0, 2, 3)
        .reshape(P, DCH * R)
    ).astype(f8dt)
    wcn = np.ascontiguousarray(
        (-WSCALE * w).reshape(DCH, P, R).transpose(1, 0, 2).reshape(P, DCH * R)
    ).astype(bf)
    wt = np.ascontiguousarray(w.T).astype(bf)

    ident = np.eye(P, dtype=bf)
    return [
        {"srcp": srcp[c], "tgtp": tgtp[c], "wc8": wc8, "wcn": wcn, "wt": wt,
         "ident": ident}
        for c in range(N_CORES)
    ]


def unpack_output(res_list, target, weight, nq=None):
    """per-core int8 [nq*128, 32*QR] -> [B, S, D] f32, with host outlier patch."""
    nq = nq or CFG["nq"]
    QR = RPC // nq
    outp = np.stack([r["outp"] for r in res_list])
    out = (
        outp.reshape(N_CORES, nq, P, DCH, QR)
        .transpose(0, 1, 4, 3, 2)
        .astype(np.float32)
        .reshape(ROWS, D)
    )
    out *= np.float32(STEP)

    # patch rows where tgt would saturate int8: out = tgt + STEP * tT @ W[d]
    tgt2 = np.asarray(target, dtype=np.float32).reshape(ROWS, D)
    w = np.asarray(weight, dtype=np.float32)
    tT = np.concatenate(
        [r["outt"].astype(np.float32).T for r in res_list], axis=0
    )  # [ROWS, R] in step units
    ii, dd = np.nonzero(np.abs(tgt2) > TPATCH)
    out[ii, dd] = tgt2[ii, dd] + np.float32(STEP) * np.einsum(
        "nr,nr->n", tT[ii], w[dd]
    )
    return np.ascontiguousarray(out.reshape(B, S, D))


LAST_RESULT = None
TRACE = False


def kernel(source, target, weight):
    from concourse.bass_utils import run_bass_kernel_spmd

    global LAST_RESULT
    in_maps = make_host_inputs(
        np.asarray(source), np.asarray(target), np.asarray(weight)
    )
    nc = _get_nc()
    res = run_bass_kernel_spmd(
        nc, in_maps, core_ids=list(range(N_CORES)), trace=TRACE
    )
    LAST_RESULT = res
    return unpack_output(res.results, target, weight)


# revision 15
# speedup vs baseline: 1.4099x; 1.4099x over previous
"""Trainium2 Bass kernel v4 for LowRankOrthogonalProjection.

    out = target + (source - target) @ W @ W.T        (W: [D, R], R=16)

All three big streams travel at 1 byte/element (25.2 MB/core HBM vs 42 MB
in v2):

  * srcp: fp8e4(src/STEP), DoubleRow-packed.  Feeds the PE directly.
  * tgtp: int8 round(tgt/STEP), plain HWDGE load; the otherwise-idle ACT
    engine upcasts to bf16 in SBUF (exact small integers), so one tile
    serves both the matmul path and the final add -- no elementwise sub
    anywhere.  (SWDGE cast-DMA would instead double the DMA-side bytes.)
  * outp: int8.  One DVE op per chunk: out_i8 = rne_sat(corr_psum + tgt_bf16)
    (DVE float->int8 output is round-nearest-even + saturating, HW-verified).

diff = src - tgt is formed *in PSUM* by a two-pass accumulation:
  pass 1: fp8 DoubleRow matmuls   src_u @ (+32 W)
  pass 2: bf16 matmuls            tgt_u @ (-32 W)
tT = 32*(diff_u @ W) -> ACT scales 1/32 -> bf16 -> m2 (wt stationary) ->
corr_u in PSUM -> DVE add+quantize -> int8 out.

Per-rep engine busy (TimelineSim, matches clean-window HW within ~5%):
DVE 76.3us (gapless bottleneck), DMA 70.4, PE 64.5, ACT 60.

The rank-16 row coefficients tT are also written out (tiny) so the host can
exactly patch the ~0.03% of rows where |tgt| > TPATCH would saturate int8.

Layout (per core, rpc=2048 rows in NQ slabs of QR=2048/NQ rows):
  srcp[q*128+p, c2*2*QR + b*QR + r] = fp8(src[q*QR+r, c2*256+2p+b]/STEP)
  tgtp[q*128+p, c*QR+r]             = int8(round(tgt[q*QR+r, c*128+p]/STEP))
  outp: same indexing as tgtp, int8.
  wc8[p, c2*32 + b*16 + j] = fp8(32*W[c2*256+2p+b, j])
  wcn[p, c*16+j]           = bf16(-32*W[c*128+p, j])
  wt[j, d]                 = bf16(W[d, j])
  outt[j, q*QR+r]          = bf16(tT_u[q*QR+r, j])
"""

import numpy as np
import ml_dtypes

B, S, D, R = 4, 4096, 4096, 16
N_CORES = 8
ROWS = B * S                 # 16384
RPC = ROWS // N_CORES        # 2048 rows per core
P = 128
DCH = D // P                 # 32 D-chunks
WSCALE = 32.0                # fp8 weight scale
STEP = 0.0325                # int8 quantization step for tgt/out
TPATCH = 3.6                 # host patches rows with |tgt| > TPATCH
_NC_CACHE = {}

# default build config (kernel() uses this; bench can override)
CFG = dict(nq=4, pb_group=2, tgt_split=2, src_split=2, out_split=2,
           tgt_bufs=2, src_bufs=4, out_bufs=2, out_dma="sync",
           tgt_upcast="act", upcast_split=8, upcast_pool=0, act_groups=0)


def build_nc(reps=1, nq=4, pb_group=2, tgt_split=2, src_split=2, out_split=2,
             tgt_bufs=2, src_bufs=4, out_bufs=2, out_dma="sync",
             tgt_upcast="dma", upcast_split=4, upcast_pool=0, act_groups=0):
    import concourse.bass as bass
    import concourse.mybir as mybir
    import concourse.tile as tile

    bf16 = mybir.dt.bfloat16
    f32 = mybir.dt.float32
    f8 = mybir.dt.float8e4
    i8 = mybir.dt.int8

    QR = RPC // nq           # rows per slab
    QW = DCH * QR            # packed free width
    HW = QW // 2             # half-slab free width
    DC2 = DCH // 2

    nc = bass.Bass("TRN2", target_bir_lowering=False)

    srcp = nc.dram_tensor("srcp", [nq * P, QW], f8, kind="ExternalInput")
    tgtp = nc.dram_tensor("tgtp", [nq * P, QW], i8, kind="ExternalInput")
    wc8 = nc.dram_tensor("wc8", [P, DCH * R], f8, kind="ExternalInput")
    wcn = nc.dram_tensor("wcn", [P, DCH * R], bf16, kind="ExternalInput")
    wt = nc.dram_tensor("wt", [R, D], bf16, kind="ExternalInput")
    outp = nc.dram_tensor("outp", [nq * P, QW], i8, kind="ExternalOutput")
    outt = nc.dram_tensor("outt", [R, RPC], bf16, kind="ExternalOutput")
    ident = nc.dram_tensor("ident", [P, P], bf16, kind="ExternalInput")

    out_eng = dict(sync="sync", scalar="scalar")[out_dma]

    with tile.TileContext(nc) as tc:
        with (
            tc.tile_pool(name="const", bufs=1) as cpool,
            tc.tile_pool(name="tgtp_", bufs=tgt_bufs) as tgt_pool,
            tc.tile_pool(name="tgt8p", bufs=2) as tgt8_pool,
            tc.tile_pool(name="srcp_", bufs=src_bufs) as src_pool,
            tc.tile_pool(name="tmtp", bufs=2) as tmt_pool,
            tc.tile_pool(name="outp_", bufs=out_bufs) as out_pool,
            tc.tile_pool(name="ps_t", bufs=2, space="PSUM") as ps_t,
            tc.tile_pool(name="ps_o", bufs=3, space="PSUM") as ps_o,
        ):
            wc8_sb = cpool.tile([P, DCH * R], f8)
            nc.sync.dma_start(wc8_sb, wc8[:, :])
            wcn_sb = cpool.tile([P, DCH * R], bf16)
            nc.sync.dma_start(wcn_sb, wcn[:, :])
            wt_sb = cpool.tile([R, D], bf16)
            nc.sync.dma_start(wt_sb, wt[:, :])
            if act_groups:
                id_sb = cpool.tile([P, P], bf16)
                nc.sync.dma_start(id_sb, ident[:, :])

            NG = DCH // 2 // pb_group * 2          # psum groups per slab
            act_sel = set(
                round(i * NG / act_groups) % NG for i in range(act_groups)
            ) if act_groups else set()

            def ap3(t2d, ko, dim):
                a = [list(x) for x in t2d.ap]
                return bass.AP(
                    t2d.tensor, t2d.offset,
                    [a[0], [dim, ko], [1, dim]],
                )

            for qi in range(nq * reps):
                q = qi % nq
                qs = q * P

                # tgt: int8 on the wire, bf16 in SBUF
                tgt_sb = tgt_pool.tile([P, QW], bf16, tag="tgt")
                tw = QW // tgt_split
                if tgt_upcast == "dma":
                    # SWDGE cast-DMA (costs 2x bytes on the SBUF fabric side)
                    for ts in range(tgt_split):
                        nc.gpsimd.dma_start(
                            tgt_sb[:, ts * tw : (ts + 1) * tw],
                            tgtp[qs : qs + P, ts * tw : (ts + 1) * tw],
                        )
                else:
                    # plain int8 load + engine upcast (pool: otherwise idle)
                    tgt8_sb = tgt8_pool.tile([P, QW], i8, tag="tgt8")
                    for ts in range(tgt_split):
                        nc.sync.dma_start(
                            tgt8_sb[:, ts * tw : (ts + 1) * tw],
                            tgtp[qs : qs + P, ts * tw : (ts + 1) * tw],
                        )
                    uw = QW // upcast_split
                    for us in range(upcast_split):
                        dst = tgt_sb[:, us * uw : (us + 1) * uw]
                        src_ = tgt8_sb[:, us * uw : (us + 1) * uw]
                        if tgt_upcast == "act":
                            if us < upcast_pool:
                                nc.gpsimd.tensor_copy(dst, src_)
                            else:
                                nc.scalar.copy(dst, src_)
                        elif tgt_upcast == "pool":
                            nc.gpsimd.tensor_copy(dst, src_)
                        else:
                            nc.vector.tensor_copy(dst, src_)

                # src: fp8 straight into SBUF (plain HWDGE)
                if src_split == 1:
                    s = src_pool.tile([P, QW], f8, tag="src")
                    nc.sync.dma_start(s, srcp[qs : qs + P, :])
                    src_sb = [s[:, 0:HW], s[:, HW:QW]]
                else:
                    src_sb = []
                    for h in range(2):
                        s = src_pool.tile([P, HW], f8, tag="src")
                        nc.sync.dma_start(
                            s, srcp[qs : qs + P, h * HW : (h + 1) * HW]
                        )
                        src_sb.append(s)

                # m1: tT_ps = 32*(src_u - tgt_u) @ W, fp8-DR + bf16 two-pass
                tT_ps = ps_t.tile([R, QR], f32, tag="tT")
                first = True
                for h in range(2):
                    for j in range(DC2 // 2):
                        c2 = h * (DC2 // 2) + j
                        nc.tensor.matmul(
                            tT_ps,
                            ap3(wc8_sb[:, c2 * 2 * R : (c2 + 1) * 2 * R], 2, R),
                            ap3(src_sb[h][:, j * 2 * QR : (j + 1) * 2 * QR], 2, QR),
                            start=first,
                            stop=False,
                            perf_mode=mybir.MatmulPerfMode.DoubleRow,
                        )
                        first = False
                    for jc in range(DCH // 2):
                        c = h * (DCH // 2) + jc
                        nc.tensor.matmul(
                            tT_ps,
                            wcn_sb[:, c * R : (c + 1) * R],
                            tgt_sb[:, c * QR : (c + 1) * QR],
                            start=False,
                            stop=(c == DCH - 1),
                        )

                tT_sb = tmt_pool.tile([R, QR], bf16, tag="tTs")
                nc.scalar.mul(tT_sb, tT_ps, 1.0 / WSCALE)
                nc.sync.dma_start(outt[:, q * QR : (q + 1) * QR], tT_sb)

                # m2 + fused add/quantize
                PB = pb_group
                OW = QW // out_split
                out_sb = None
                ob = 0
                gidx = 0
                for h in range(2):
                    if out_sb is None:
                        out_sb = out_pool.tile([P, OW], i8, tag="out")
                        ob = h * (DCH // 2)
                    for j in range(DCH // 2 // PB):
                        use_act = gidx in act_sel
                        gidx += 1
                        o_ps = ps_o.tile([P, PB * QR], f32, tag="ops")
                        for b in range(PB):
                            c = h * (DCH // 2) + j * PB + b
                            nc.tensor.matmul(
                                o_ps[:, b * QR : (b + 1) * QR],
                                wt_sb[:, c * P : (c + 1) * P],
                                tT_sb,
                                start=True,
                                stop=not use_act,
                                skip_group_check=use_act,
                            )
                        c0 = h * (DCH // 2) + j * PB
                        if use_act:
                            # fold tgt into PSUM on the PE, evacuate on ACT
                            for b in range(PB):
                                nc.tensor.matmul(
                                    o_ps[:, b * QR : (b + 1) * QR],
                                    id_sb,
                                    tgt_sb[:, (c0 + b) * QR : (c0 + b + 1) * QR],
                                    start=False,
                                    stop=(b == PB - 1),
                                    skip_group_check=True,
                                )
                            nc.scalar.copy(
                                out_sb[:, (c0 - ob) * QR : (c0 - ob + PB) * QR],
                                o_ps,
                            )
                        else:
                            nc.vector.tensor_add(
                                out_sb[:, (c0 - ob) * QR : (c0 - ob + PB) * QR],
                                o_ps,
                                tgt_sb[:, c0 * QR : (c0 + PB) * QR],
                            )
                    if (h + 1) % (2 // out_split) == 0:
                        getattr(nc, out_eng).dma_start(
                            outp[qs : qs + P, ob * QR : ob * QR + OW], out_sb
                        )
                        out_sb = None

    return nc


def split_waits(nc, limit=1):
    """Walrus encodes at most one semaphore wait per instruction; hoist
    extras onto standalone EventSemaphore instructions."""
    import concourse.mybir as mybir

    nsplit = 0
    for fn in nc.m.functions:
        for blk in fn.blocks:
            new = []
            for ins in blk.instructions:
                si = ins.sync_info
                waits = list(si.on_wait) if si is not None and si.on_wait else []
                if len(waits) > limit:
                    for k, w in enumerate(waits[:-limit]):
                        es = mybir.InstEventSemaphore(
                            name=f"{ins.name}-hw{k}",
                            engine=ins.engine,
                            sync_info=mybir.SyncInfo(on_wait=[w], on_update=[]),
                        )
                        new.append(es)
                        nsplit += 1
                    ins.sync_info = mybir.SyncInfo(
                        on_wait=waits[-limit:],
                        on_update=list(si.on_update or []),
                    )
                new.append(ins)
            blk.instructions[:] = new
    return nsplit


def _get_nc(reps=1, raw=False, **kw):
    cfg = dict(CFG)
    cfg.update(kw)
    key = (reps, raw, tuple(sorted(cfg.items())))
    if key not in _NC_CACHE:
        nc = build_nc(reps, **cfg)
        nc.finalize()
        if not raw:
            split_waits(nc)
        _NC_CACHE[key] = nc
    return _NC_CACHE[key]


def _pack(x2, dtype, nq):
    """[16384, 4096] row-major -> per-core [nq*128, 32*QR] slab-packed."""
    QR = RPC // nq
    xq = np.asarray(x2).astype(dtype)
    xp = (
        xq.reshape(N_CORES, nq, QR, DCH, P)
        .transpose(0, 1, 4, 3, 2)
        .reshape(N_CORES, nq * P, DCH * QR)
    )
    return np.ascontiguousarray(xp)


def _pack_dr(x2, dtype, nq):
    """DoubleRow pack: [q*128+p, c2*2*QR + b*QR + r] = x[q*QR+r, c2*256+2p+b]."""
    QR = RPC // nq
    DC2 = DCH // 2
    xq = np.asarray(x2).astype(dtype)
    xp = (
        xq.reshape(N_CORES, nq, QR, DC2, P, 2)     # (core, q, r, c2, p, b)
        .transpose(0, 1, 4, 3, 5, 2)               # (core, q, p, c2, b, r)
        .reshape(N_CORES, nq * P, DCH * QR)
    )
    return np.ascontiguousarray(xp)


def make_host_inputs(source, target, weight, nq=None):
    import concourse.mybir as mybir

    nq = nq or CFG["nq"]
    f8dt = mybir.dt.np(mybir.dt.float8e4)
    bf = ml_dtypes.bfloat16

    src2 = np.asarray(source, dtype=np.float32).reshape(ROWS, D)
    tgt2 = np.asarray(target, dtype=np.float32).reshape(ROWS, D)
    w = np.asarray(weight, dtype=np.float32)

    srcp = _pack_dr(src2 * np.float32(1.0 / STEP), f8dt, nq)
    tgtq = np.clip(np.rint(tgt2 * np.float32(1.0 / STEP)), -127, 127).astype(
        np.int8
    )
    tgtp = _pack(tgtq, np.int8, nq)
    # wc8[p, c2*32 + b*16 + j] = 32*W[c2*256 + 2p + b, j]
    wc8 = np.ascontiguousarray(
        np.clip(WSCALE * w, -240, 240)
        .reshape(DCH // 2, P, 2, R)
        .transpose(1, 0, 2, 3)
        .reshape(P, DCH * R)
    ).astype(f8dt)
    wcn = np.ascontiguousarray(
        (-WSCALE * w).reshape(DCH, P, R).transpose(1, 0, 2).reshape(P, DCH * R)
    ).astype(bf)
    wt = np.ascontiguousarray(w.T).astype(bf)

    ident = np.eye(P, dtype=bf)
    return [
        {"srcp": srcp[c], "tgtp": tgtp[c], "wc8": wc8, "wcn": wcn, "wt": wt,
         "ident": ident}
        for c in range(N_CORES)
    ]


def unpack_output(res_list, target, weight, nq=None):
    """per-core int8 [nq*128, 32*QR] -> [B, S, D] f32, with host outlier patch."""
    nq = nq or CFG["nq"]
    QR = RPC // nq
    outp = np.stack([r["outp"] for r in res_list])
    out = (
        outp.reshape(N_CORES, nq, P, DCH, QR)
        .transpose(0, 1, 4, 3, 2)
        .astype(np.float32)
        .reshape(ROWS, D)
    )
    out *= np.float32(STEP)

    # patch rows where tgt would saturate int8: out = tgt + STEP * tT @ W[d]
    tgt2 = np.asarray(target, dtype=np.float32).reshape(ROWS, D)
    w = np.asarray(weight, dtype=np.float32)
    tT = np.concatenate(
        [r["outt"].astype(np.float32).T for r in res_list], axis=0
    )  # [ROWS, R] in step units
    ii, dd = np.nonzero(np.abs(tgt2) > TPATCH)
    out[ii, dd] = tgt2[ii, dd] + np.float32(STEP) * np.einsum(
        "nr,nr->n", tT[ii], w[dd]
    )
    return np.ascontiguousarray(out.reshape(B, S, D))


LAST_RESULT = None
TRACE = False


def kernel(source, target, weight):
    from concourse.bass_utils import run_bass_kernel_spmd

    global LAST_RESULT
    in_maps = make_host_inputs(
        np.asarray(source), np.asarray(target), np.asarray(weight)
    )
    nc = _get_nc()
    res = run_bass_kernel_spmd(
        nc, in_maps, core_ids=list(range(N_CORES)), trace=TRACE
    )
    LAST_RESULT = res
    return unpack_output(res.results, target, weight)


# revision 19
# speedup vs baseline: 1.4795x; 1.0494x over previous
"""Trainium2 Bass kernel v4 for LowRankOrthogonalProjection.

    out = target + (source - target) @ W @ W.T        (W: [D, R], R=16)

All three big streams travel at 1 byte/element (25.2 MB/core HBM vs 42 MB
in v2):

  * srcp: fp8e4(src/STEP), DoubleRow-packed.  Feeds the PE directly.
  * tgtp: int8 round(tgt/STEP), plain HWDGE load; the otherwise-idle ACT
    engine upcasts to bf16 in SBUF (exact small integers), so one tile
    serves both the matmul path and the final add -- no elementwise sub
    anywhere.  (SWDGE cast-DMA would instead double the DMA-side bytes.)
  * outp: int8.  One DVE op per chunk: out_i8 = rne_sat(corr_psum + tgt_bf16)
    (DVE float->int8 output is round-nearest-even + saturating, HW-verified).

diff = src - tgt is formed *in PSUM* by a two-pass accumulation:
  pass 1: fp8 DoubleRow matmuls   src_u @ (+32 W)
  pass 2: bf16 matmuls            tgt_u @ (-32 W)
tT = 32*(diff_u @ W) -> ACT scales 1/32 -> bf16 -> m2 (wt stationary) ->
corr_u in PSUM -> DVE add+quantize -> int8 out.

Per-rep engine busy (TimelineSim, matches clean-window HW within ~5%):
DVE 76.3us (gapless bottleneck), DMA 70.4, PE 64.5, ACT 60.

The rank-16 row coefficients tT are also written out (tiny) so the host can
exactly patch the ~0.03% of rows where |tgt| > TPATCH would saturate int8.

Layout (per core, rpc=2048 rows in NQ slabs of QR=2048/NQ rows):
  srcp[q*128+p, c2*2*QR + b*QR + r] = fp8(src[q*QR+r, c2*256+2p+b]/STEP)
  tgtp[q*128+p, c*QR+r]             = int8(round(tgt[q*QR+r, c*128+p]/STEP))
  outp: same indexing as tgtp, int8.
  wc8[p, c2*32 + b*16 + j] = fp8(32*W[c2*256+2p+b, j])
  wcn[p, c*16+j]           = bf16(-32*W[c*128+p, j])
  wt[j, d]                 = bf16(W[d, j])
  outt[j, q*QR+r]          = bf16(tT_u[q*QR+r, j])
"""

import numpy as np
import ml_dtypes

B, S, D, R = 4, 4096, 4096, 16
N_CORES = 8
ROWS = B * S                 # 16384
RPC = ROWS // N_CORES        # 2048 rows per core
P = 128
DCH = D // P                 # 32 D-chunks
WSCALE = 32.0                # fp8 weight scale
STEP = 0.0325                # int8 quantization step for tgt/out
TPATCH = 3.6                 # host patches rows with |tgt| > TPATCH
_NC_CACHE = {}

# default build config (kernel() uses this; bench can override)
CFG = dict(nq=4, pb_group=2, tgt_split=2, src_split=2, out_split=2,
           tgt_bufs=2, src_bufs=4, out_bufs=2, out_dma="sync",
           tgt_upcast="act", upcast_split=8, upcast_pool=0, act_groups=0,
           pipeline=True)


def build_nc(reps=1, nq=4, pb_group=2, tgt_split=2, src_split=2, out_split=2,
             tgt_bufs=2, src_bufs=4, out_bufs=2, out_dma="sync",
             tgt_upcast="dma", upcast_split=4, upcast_pool=0, act_groups=0,
             pipeline=False):
    import concourse.bass as bass
    import concourse.mybir as mybir
    import concourse.tile as tile

    bf16 = mybir.dt.bfloat16
    f32 = mybir.dt.float32
    f8 = mybir.dt.float8e4
    i8 = mybir.dt.int8

    QR = RPC // nq           # rows per slab
    QW = DCH * QR            # packed free width
    HW = QW // 2             # half-slab free width
    DC2 = DCH // 2

    nc = bass.Bass("TRN2", target_bir_lowering=False)

    srcp = nc.dram_tensor("srcp", [nq * P, QW], f8, kind="ExternalInput")
    tgtp = nc.dram_tensor("tgtp", [nq * P, QW], i8, kind="ExternalInput")
    wc8 = nc.dram_tensor("wc8", [P, DCH * R], f8, kind="ExternalInput")
    wcn = nc.dram_tensor("wcn", [P, DCH * R], bf16, kind="ExternalInput")
    wt = nc.dram_tensor("wt", [R, D], bf16, kind="ExternalInput")
    outp = nc.dram_tensor("outp", [nq * P, QW], i8, kind="ExternalOutput")
    outt = nc.dram_tensor("outt", [R, RPC], bf16, kind="ExternalOutput")
    ident = nc.dram_tensor("ident", [P, P], bf16, kind="ExternalInput")

    out_eng = dict(sync="sync", scalar="scalar")[out_dma]

    with tile.TileContext(nc) as tc:
        with (
            tc.tile_pool(name="const", bufs=1) as cpool,
            tc.tile_pool(name="tgtp_", bufs=tgt_bufs) as tgt_pool,
            tc.tile_pool(name="tgt8p", bufs=2) as tgt8_pool,
            tc.tile_pool(name="srcp_", bufs=src_bufs) as src_pool,
            tc.tile_pool(name="tmtp", bufs=2) as tmt_pool,
            tc.tile_pool(name="outp_", bufs=out_bufs) as out_pool,
            tc.tile_pool(name="ps_t", bufs=2, space="PSUM") as ps_t,
            tc.tile_pool(name="ps_o", bufs=3, space="PSUM") as ps_o,
        ):
            wc8_sb = cpool.tile([P, DCH * R], f8)
            nc.sync.dma_start(wc8_sb, wc8[:, :])
            wcn_sb = cpool.tile([P, DCH * R], bf16)
            nc.sync.dma_start(wcn_sb, wcn[:, :])
            wt_sb = cpool.tile([R, D], bf16)
            nc.sync.dma_start(wt_sb, wt[:, :])
            if act_groups:
                id_sb = cpool.tile([P, P], bf16)
                nc.sync.dma_start(id_sb, ident[:, :])

            NG = DCH // 2 // pb_group * 2          # psum groups per slab
            act_sel = set(
                round(i * NG / act_groups) % NG for i in range(act_groups)
            ) if act_groups else set()

            def ap3(t2d, ko, dim):
                a = [list(x) for x in t2d.ap]
                return bass.AP(
                    t2d.tensor, t2d.offset,
                    [a[0], [dim, ko], [1, dim]],
                )

            PB = pb_group
            OW = QW // out_split

            def front(q):
                """Loads + upcast + m1 spec list for slab q."""
                qs = q * P
                tgt_sb = tgt_pool.tile([P, QW], bf16, tag="tgt")
                tw = QW // tgt_split
                if tgt_upcast == "dma":
                    for ts in range(tgt_split):
                        nc.gpsimd.dma_start(
                            tgt_sb[:, ts * tw : (ts + 1) * tw],
                            tgtp[qs : qs + P, ts * tw : (ts + 1) * tw],
                        )
                else:
                    tgt8_sb = tgt8_pool.tile([P, QW], i8, tag="tgt8")
                    for ts in range(tgt_split):
                        nc.sync.dma_start(
                            tgt8_sb[:, ts * tw : (ts + 1) * tw],
                            tgtp[qs : qs + P, ts * tw : (ts + 1) * tw],
                        )
                    uw = QW // upcast_split
                    for us in range(upcast_split):
                        dst = tgt_sb[:, us * uw : (us + 1) * uw]
                        src_ = tgt8_sb[:, us * uw : (us + 1) * uw]
                        if tgt_upcast == "act":
                            if us < upcast_pool:
                                nc.gpsimd.tensor_copy(dst, src_)
                            else:
                                nc.scalar.copy(dst, src_)
                        elif tgt_upcast == "pool":
                            nc.gpsimd.tensor_copy(dst, src_)
                        else:
                            nc.vector.tensor_copy(dst, src_)

                if src_split == 1:
                    s = src_pool.tile([P, QW], f8, tag="src")
                    nc.sync.dma_start(s, srcp[qs : qs + P, :])
                    src_sb = [s[:, 0:HW], s[:, HW:QW]]
                else:
                    src_sb = []
                    for h in range(2):
                        s = src_pool.tile([P, HW], f8, tag="src")
                        nc.sync.dma_start(
                            s, srcp[qs : qs + P, h * HW : (h + 1) * HW]
                        )
                        src_sb.append(s)

                tT_ps = ps_t.tile([R, QR], f32, tag="tT")
                specs = []
                for h in range(2):
                    for j in range(DC2 // 2):
                        specs.append(("dr", h, j))
                    for jc in range(DCH // 2):
                        specs.append(("tg", h, jc))
                return dict(q=q, qs=qs, tgt=tgt_sb, src=src_sb,
                            tT_ps=tT_ps, specs=specs)

            def emit_m1(cur, idx):
                kind, h, j = cur["specs"][idx]
                n = len(cur["specs"])
                start, stop = idx == 0, idx == n - 1
                if kind == "dr":
                    c2 = h * (DC2 // 2) + j
                    nc.tensor.matmul(
                        cur["tT_ps"],
                        ap3(wc8_sb[:, c2 * 2 * R : (c2 + 1) * 2 * R], 2, R),
                        ap3(cur["src"][h][:, j * 2 * QR : (j + 1) * 2 * QR], 2, QR),
                        start=start,
                        stop=stop,
                        perf_mode=mybir.MatmulPerfMode.DoubleRow,
                        skip_group_check=pipeline,
                    )
                else:
                    c = h * (DCH // 2) + j
                    nc.tensor.matmul(
                        cur["tT_ps"],
                        wcn_sb[:, c * R : (c + 1) * R],
                        cur["tgt"][:, c * QR : (c + 1) * QR],
                        start=start,
                        stop=stop,
                        skip_group_check=pipeline,
                    )

            def back(cur):
                """tT copy + outt store for slab cur (after its m1)."""
                tT_sb = tmt_pool.tile([R, QR], bf16, tag="tTs")
                nc.scalar.mul(tT_sb, cur["tT_ps"], 1.0 / WSCALE)
                nc.sync.dma_start(
                    outt[:, cur["q"] * QR : (cur["q"] + 1) * QR], tT_sb
                )
                cur["tT_sb"] = tT_sb

            def emit_m2_unit(prev, gidx, state):
                """One psum group of slab prev: PB matmuls + add/evac (+store)."""
                h, j = divmod(gidx, DCH // 2 // PB)
                if state.get("out") is None:
                    out_tile = out_pool.tile([P, OW], i8, tag="out")
                    state["out"] = out_tile
                    state["ob"] = h * (DCH // 2)
                out_sb, ob = state["out"], state["ob"]
                use_act = gidx in act_sel
                o_ps = ps_o.tile([P, PB * QR], f32, tag="ops")
                for b in range(PB):
                    c = h * (DCH // 2) + j * PB + b
                    nc.tensor.matmul(
                        o_ps[:, b * QR : (b + 1) * QR],
                        wt_sb[:, c * P : (c + 1) * P],
                        prev["tT_sb"],
                        start=True,
                        stop=not use_act,
                        skip_group_check=use_act or pipeline,
                    )
                c0 = h * (DCH // 2) + j * PB
                if use_act:
                    for b in range(PB):
                        nc.tensor.matmul(
                            o_ps[:, b * QR : (b + 1) * QR],
                            id_sb,
                            prev["tgt"][:, (c0 + b) * QR : (c0 + b + 1) * QR],
                            start=False,
                            stop=(b == PB - 1),
                            skip_group_check=True,
                        )
                    nc.scalar.copy(
                        out_sb[:, (c0 - ob) * QR : (c0 - ob + PB) * QR], o_ps
                    )
                else:
                    nc.vector.tensor_add(
                        out_sb[:, (c0 - ob) * QR : (c0 - ob + PB) * QR],
                        o_ps,
                        prev["tgt"][:, c0 * QR : (c0 + PB) * QR],
                    )
                last_of_h = (j + 1) == DCH // 2 // PB
                if last_of_h and (h + 1) % (2 // out_split) == 0:
                    getattr(nc, out_eng).dma_start(
                        outp[prev["qs"] : prev["qs"] + P,
                             ob * QR : ob * QR + OW],
                        out_sb,
                    )
                    state["out"] = None

            NU = DCH // 2 // PB * 2            # m2 units per slab
            if pipeline:
                prev = None
                for qi in range(nq * reps + 1):
                    cur = front(qi % nq) if qi < nq * reps else None
                    n1 = len(cur["specs"]) if cur else 0
                    n2 = NU if prev else 0
                    state = {}
                    i1 = 0
                    for i2 in range(n2):
                        quota = ((i2 + 1) * n1) // n2
                        while i1 < quota:
                            emit_m1(cur, i1)
                            i1 += 1
                        emit_m2_unit(prev, i2, state)
                    while i1 < n1:
                        emit_m1(cur, i1)
                        i1 += 1
                    if cur:
                        back(cur)
                    prev = cur
            else:
                for qi in range(nq * reps):
                    cur = front(qi % nq)
                    for i1 in range(len(cur["specs"])):
                        emit_m1(cur, i1)
                    back(cur)
                    state = {}
                    for gidx in range(NU):
                        emit_m2_unit(cur, gidx, state)

    return nc


def split_waits(nc, limit=1):
    """Walrus encodes at most one semaphore wait per instruction; hoist
    extras onto standalone EventSemaphore instructions."""
    import concourse.mybir as mybir

    nsplit = 0
    for fn in nc.m.functions:
        for blk in fn.blocks:
            new = []
            for ins in blk.instructions:
                si = ins.sync_info
                waits = list(si.on_wait) if si is not None and si.on_wait else []
                if len(waits) > limit:
                    for k, w in enumerate(waits[:-limit]):
                        es = mybir.InstEventSemaphore(
                            name=f"{ins.name}-hw{k}",
                            engine=ins.engine,
                            sync_info=mybir.SyncInfo(on_wait=[w], on_update=[]),
                        )
                        new.append(es)
                        nsplit += 1
                    ins.sync_info = mybir.SyncInfo(
                        on_wait=waits[-limit:],
                        on_update=list(si.on_update or []),
                    )
                new.append(ins)
            blk.instructions[:] = new
    return nsplit


def _get_nc(reps=1, raw=False, **kw):
    cfg = dict(CFG)
    cfg.update(kw)
    key = (reps, raw, tuple(sorted(cfg.items())))
    if key not in _NC_CACHE:
        nc = build_nc(reps, **cfg)
        nc.finalize()
        if not raw:
            split_waits(nc)
        _NC_CACHE[key] = nc
    return _NC_CACHE[key]


def _pack(x2, dtype, nq):
    """[16384, 4096] row-major -> per-core [nq*128, 32*QR] slab-packed."""
    QR = RPC // nq
    xq = np.asarray(x2).astype(dtype)
    xp = (
        xq.reshape(N_CORES, nq, QR, DCH, P)
        .transpose(0, 1, 4, 3, 2)
        .reshape(N_CORES, nq * P, DCH * QR)
    )
    return np.ascontiguousarray(xp)


def _pack_dr(x2, dtype, nq):
    """DoubleRow pack: [q*128+p, c2*2*QR + b*QR + r] = x[q*QR+r, c2*256+2p+b]."""
    QR = RPC // nq
    DC2 = DCH // 2
    xq = np.asarray(x2).astype(dtype)
    xp = (
        xq.reshape(N_CORES, nq, QR, DC2, P, 2)     # (core, q, r, c2, p, b)
        .transpose(0, 1, 4, 3, 5, 2)               # (core, q, p, c2, b, r)
        .reshape(N_CORES, nq * P, DCH * QR)
    )
    return np.ascontiguousarray(xp)


def make_host_inputs(source, target, weight, nq=None):
    import concourse.mybir as mybir

    nq = nq or CFG["nq"]
    f8dt = mybir.dt.np(mybir.dt.float8e4)
    bf = ml_dtypes.bfloat16

    src2 = np.asarray(source, dtype=np.float32).reshape(ROWS, D)
    tgt2 = np.asarray(target, dtype=np.float32).reshape(ROWS, D)
    w = np.asarray(weight, dtype=np.float32)

    srcp = _pack_dr(src2 * np.float32(1.0 / STEP), f8dt, nq)
    tgtq = np.clip(np.rint(tgt2 * np.float32(1.0 / STEP)), -127, 127).astype(
        np.int8
    )
    tgtp = _pack(tgtq, np.int8, nq)
    # wc8[p, c2*32 + b*16 + j] = 32*W[c2*256 + 2p + b, j]
    wc8 = np.ascontiguousarray(
        np.clip(WSCALE * w, -240, 240)
        .reshape(DCH // 2, P, 2, R)
        .transpose(1, 0, 2, 3)
        .reshape(P, DCH * R)
    ).astype(f8dt)
    wcn = np.ascontiguousarray(
        (-WSCALE * w).reshape(DCH, P, R).transpose(1, 0, 2).reshape(P, DCH * R)
    ).astype(bf)
    wt = np.ascontiguousarray(w.T).astype(bf)

    ident = np.eye(P, dtype=bf)
    return [
        {"srcp": srcp[c], "tgtp": tgtp[c], "wc8": wc8, "wcn": wcn, "wt": wt,
         "ident": ident}
        for c in range(N_CORES)
    ]


def unpack_output(res_list, target, weight, nq=None):
    """per-core int8 [nq*128, 32*QR] -> [B, S, D] f32, with host outlier patch."""
    nq = nq or CFG["nq"]
    QR = RPC // nq
    outp = np.stack([r["outp"] for r in res_list])
    out = (
        outp.reshape(N_CORES, nq, P, DCH, QR)
        .transpose(0, 1, 4, 3, 2)
        .astype(np.float32)
        .reshape(ROWS, D)
    )
    out *= np.float32(STEP)

    # patch rows where tgt would saturate int8: out = tgt + STEP * tT @ W[d]
    tgt2 = np.asarray(target, dtype=np.float32).reshape(ROWS, D)
    w = np.asarray(weight, dtype=np.float32)
    tT = np.concatenate(
        [r["outt"].astype(np.float32).T for r in res_list], axis=0
    )  # [ROWS, R] in step units
    ii, dd = np.nonzero(np.abs(tgt2) > TPATCH)
    out[ii, dd] = tgt2[ii, dd] + np.float32(STEP) * np.einsum(
        "nr,nr->n", tT[ii], w[dd]
    )
    return np.ascontiguousarray(out.reshape(B, S, D))


LAST_RESULT = None
TRACE = False


def kernel(source, target, weight):
    from concourse.bass_utils import run_bass_kernel_spmd

    global LAST_RESULT
    in_maps = make_host_inputs(
        np.asarray(source), np.asarray(target), np.asarray(weight)
    )
    nc = _get_nc()
    res = run_bass_kernel_spmd(
        nc, in_maps, core_ids=list(range(N_CORES)), trace=TRACE
    )
    LAST_RESULT = res
    return unpack_output(res.results, target, weight)
